# revision 1
# baseline (speedup 1.0000x reference)
"""Trainium2 Bass kernel: chunked sliding-window attention block (8-core SPMD).

Model (reference): q/k/v/o projections (1024->1024, 16 heads x 64) + causal
sliding-window attention, window=128, over x [2, 4096, 1024] fp32.

Sharding: 8 shards over (batch, seq): core c -> batch c//4, positions
[(c%4)*1024, +1024). Each core also receives a 128-position halo of x (the
previous window) so K/V for the left edge are recomputed locally; the first
shard of each batch gets zeros + a mask that blocks the halo block.

Per-core pipeline (all on one NeuronCore, built with Tile):
  phase 1: K^T = Wk @ X^T, Q^T = Wq @ X^T (fp32r, [dims, pos] layout),
           V = X @ Wv^T (natural [pos, dims], stored bf16)
  phase 2: per chunk of 128 queries, per head:
           S = Q^T.T @ K^T-window (one fp32r matmul, N=256)
           S' = S*0.125 + mask (VectorE), P = exp(S') (ScalarE, bf16,
           row-sums via accum_out), P^T via PE transpose,
           O = P^T.T @ V-blocks (bf16), A = O * 1/rowsum (VectorE)
           then A^T via PE transpose and out = A^T.T @ Wo^T (bf16),
           DMA out (fp32).
"""
import numpy as np
import ml_dtypes

import concourse.bass as bass
import concourse.tile as tile
from concourse import bacc, mybir
from concourse.bass_utils import run_bass_kernel_spmd
from concourse.masks import make_identity

B = 2
S = 4096
D = 1024
H = 16
DH = 64
W = 128
N_CORES = 8
SHARDS_PER_B = N_CORES // B
OWN = S // SHARDS_PER_B          # 1024 positions per core
HALO = W                          # 128
LOC = OWN + HALO                  # 1152 positions incl. halo
NCHUNK = OWN // W                 # 8 query chunks per core
MASK_NEG = -30000.0

F32 = mybir.dt.float32
F32R = mybir.dt.float32r
BF16 = mybir.dt.bfloat16


def build_nc(reps: int = 1):
    nc = bacc.Bacc()
    xt_d = nc.declare_dram_parameter("xt", [D, LOC], F32R, isOutput=False)
    wqt_d = nc.declare_dram_parameter("wqt", [D, D], F32R, isOutput=False)
    wkt_d = nc.declare_dram_parameter("wkt", [D, D], F32R, isOutput=False)
    wvt_d = nc.declare_dram_parameter("wvt", [D, D], F32R, isOutput=False)
    wot_d = nc.declare_dram_parameter("wot", [D, D], mybir.dt.bfloat16, isOutput=False)
    masks_d = nc.declare_dram_parameter("masks", [3, W, 2 * W], F32R, isOutput=False)
    ident_d = nc.declare_dram_parameter("ident", [128, 128], F32R, isOutput=False)
    out_d = nc.declare_dram_parameter("out", [OWN, D], F32, isOutput=True)

    with tile.TileContext(nc) as tc:
        if reps == 1:
            _build_rep(nc, tc, xt_d, wqt_d, wkt_d, wvt_d, wot_d, masks_d, ident_d, out_d)
        else:
            engines = [mybir.EngineType.PE, mybir.EngineType.Activation,
                       mybir.EngineType.DVE, mybir.EngineType.SP,
                       mybir.EngineType.Pool]
            with tc.For_i(0, reps, 1, hint_engines=tuple(engines)):
                _build_rep(nc, tc, xt_d, wqt_d, wkt_d, wvt_d, wot_d, masks_d, ident_d, out_d)
    nc.compile()
    return nc


def _build_rep(nc, tc, xt_d, wqt_d, wkt_d, wvt_d, wot_d, masks_d, ident_d, out_d):
    import contextlib
    with contextlib.ExitStack() as stk:
        const = stk.enter_context(tc.tile_pool(name="const", bufs=1))
        qt_p = stk.enter_context(tc.tile_pool(name="qt", bufs=1))
        kt_p = stk.enter_context(tc.tile_pool(name="kt", bufs=1))
        v_p = stk.enter_context(tc.tile_pool(name="v", bufs=1))

        masks_sb = const.tile([W, 3, 2 * W], F32R)
        for mi in range(3):
            nc.sync.dma_start(out=masks_sb[:, mi, :], in_=masks_d[mi])
        ident_r = const.tile([128, 128], F32R)
        nc.sync.dma_start(out=ident_r, in_=ident_d[:, :])
        ident_bf = const.tile([128, 128], BF16)
        make_identity(nc, ident_bf)
        wo_sb = const.tile([128, 8, D], BF16)
        for j in range(8):
            nc.sync.dma_start(out=wo_sb[:, j, :], in_=wot_d[j * 128:(j + 1) * 128, :])

        qt_sb = qt_p.tile([128, 8, OWN], F32R)
        kt_sb = kt_p.tile([128, 8, LOC], F32R)
        v_sb = v_p.tile([128, LOC // W, H, 68], BF16)

        # ---------------- phase 1: projections ----------------
        with tc.tile_pool(name="xtw", bufs=1) as xtw, \
             tc.tile_pool(name="wslot", bufs=2) as wslot, \
             tc.tile_pool(name="proj_ps", bufs=4, space="PSUM") as proj_ps:
            xt_sb = xtw.tile([128, 8, LOC], F32R)
            for k in range(8):
                nc.sync.dma_start(out=xt_sb[:, k, :], in_=xt_d[k * 128:(k + 1) * 128, :])

            def load_w(dram):
                w_sb = wslot.tile([128, 8, D], F32R, tag="w")
                for k in range(8):
                    nc.sync.dma_start(out=w_sb[:, k, :], in_=dram[k * 128:(k + 1) * 128, :])
                return w_sb

            # K^T [dims, all LOC positions]
            wk_sb = load_w(wkt_d)
            for m in range(8):
                for off, width in ((0, 512), (512, 512), (1024, 128)):
                    p = proj_ps.tile([128, 512], F32, tag="pp")
                    for k in range(8):
                        nc.tensor.matmul(
                            p[:, :width],
                            wk_sb[:, k, m * 128:(m + 1) * 128],
                            xt_sb[:, k, off:off + width],
                            start=(k == 0), stop=(k == 7),
                        )
                    nc.vector.tensor_copy(kt_sb[:, m, off:off + width], p[:, :width])

            # Q^T [dims, own positions]
            wq_sb = load_w(wqt_d)
            for m in range(8):
                for n in range(2):
                    p = proj_ps.tile([128, 512], F32, tag="pp")
                    for k in range(8):
                        nc.tensor.matmul(
                            p,
                            wq_sb[:, k, m * 128:(m + 1) * 128],
                            xt_sb[:, k, HALO + n * 512:HALO + (n + 1) * 512],
                            start=(k == 0), stop=(k == 7),
                        )
                    nc.vector.tensor_scalar_mul(qt_sb[:, m, n * 512:(n + 1) * 512], p, 0.125)

            # V natural [pos, dims] (bf16)
            wv_sb = load_w(wvt_d)
            for t in range(LOC // W):
                nc.vector.memset(v_sb[:, t, :, 64:65], 1.0)
                for n in range(2):
                    p = proj_ps.tile([128, 512], F32, tag="pp")
                    for k in range(8):
                        nc.tensor.matmul(
                            p,
                            xt_sb[:, k, t * 128:(t + 1) * 128],
                            wv_sb[:, k, n * 512:(n + 1) * 512],
                            start=(k == 0), stop=(k == 7),
                        )
                    nc.vector.tensor_copy(
                        v_sb[:, t, n * 8:(n + 1) * 8, 0:64],
                        p.rearrange("p (h d) -> p h d", h=8))

        # ---------------- phase 2: attention + out-projection ----------------
        with tc.tile_pool(name="attn_sb", bufs=6) as asb, \
             tc.tile_pool(name="ptb_sb", bufs=5) as ptb_p, \
             tc.tile_pool(name="a_sb", bufs=3) as apool, \
             tc.tile_pool(name="ob_sb", bufs=2) as obp, \
             tc.tile_pool(name="sc_ps", bufs=4, space="PSUM") as sc_ps, \
             tc.tile_pool(name="tr_ps", bufs=1, space="PSUM") as tr_ps, \
             tc.tile_pool(name="pv_ps", bufs=2, space="PSUM") as pv_ps, \
             tc.tile_pool(name="out_ps", bufs=1, space="PSUM") as out_ps:
            ptbs = {}
            a_cs = {}
            for t in range(NCHUNK + 3):
                if t <= NCHUNK:
                    qlo = max(t - 1, 0) * W
                    qhi = min(t + 1, NCHUNK) * W
                    width = qhi - qlo
                    dst0 = W if t == 0 else 0
                    mask_i = 0 if t == 0 else (2 if t == NCHUNK else 1)
                    ptb = ptb_p.tile([128, H, 2 * W], BF16, tag="ptb")
                    ptbs[t] = ptb
                    for pr in range(8):
                        sc2 = sc_ps.tile([128, 2, 2 * W], F32, tag="sc")
                        for i in range(2):
                            h = 2 * pr + i
                            pb = (h % 2) * 64
                            rb = h // 2
                            nc.tensor.matmul(
                                sc2[:, i, dst0:dst0 + width], ident_r,
                                masks_sb[:, mask_i, dst0:dst0 + width],
                                start=True, stop=False,
                            )
                            nc.tensor.matmul(
                                sc2[:, i, dst0:dst0 + width],
                                kt_sb[pb:pb + 64, rb, t * W:(t + 1) * W],
                                qt_sb[pb:pb + 64, rb, qlo:qhi],
                                start=False, stop=True,
                            )
                        nc.scalar.activation(
                            ptb[:, 2 * pr:2 * pr + 2, dst0:dst0 + width],
                            sc2[:, :, dst0:dst0 + width],
                            mybir.ActivationFunctionType.Exp,
                        )
                if 2 <= t <= NCHUNK + 1:
                    c = t - 2
                    a_c = apool.tile([W, D], BF16, tag="a")
                    a_cs[c] = a_c
                    pp, pc = ptbs[c], ptbs[c + 1]
                    for pr in range(8):
                        if pr % 2 == 0:
                            pv4 = pv_ps.tile([W, 4, 128], F32, tag="pv")
                        for i in range(2):
                            h = 2 * pr + i
                            s = (pr % 2) * 2 + i
                            nc.tensor.matmul(
                                pv4[:, s, 0:65],
                                pp[:, h, W:2 * W],
                                v_sb[:, c, h, 0:65],
                                start=True, stop=False,
                            )
                            nc.tensor.matmul(
                                pv4[:, s, 0:65],
                                pc[:, h, 0:W],
                                v_sb[:, c + 1, h, 0:65],
                                start=False, stop=True,
                            )
                        rd2 = asb.tile([W, 2], F32, tag="rd")
                        for i in range(2):
                            s = (pr % 2) * 2 + i
                            nc.vector.reciprocal(rd2[:, i:i + 1], pv4[:, s, 64:65])
                        for i in range(2):
                            h = 2 * pr + i
                            s = (pr % 2) * 2 + i
                            nc.vector.tensor_scalar_mul(
                                a_c[:, h * DH:(h + 1) * DH], pv4[:, s, 0:64],
                                rd2[:, i:i + 1])
                if 3 <= t <= NCHUNK + 2:
                    c = t - 3
                    a_c = a_cs.pop(c)
                    at = asb.tile([128, 8, W], BF16, tag="at")
                    for j in range(8):
                        tp = tr_ps.tile([128, 128], BF16, tag="tp")
                        nc.tensor.transpose(tp, a_c[:, j * 128:(j + 1) * 128], ident_bf)
                        nc.vector.tensor_copy(at[:, j, :], tp)
                    for n in range(2):
                        op = out_ps.tile([128, 512], F32, tag="op")
                        for j in range(8):
                            nc.tensor.matmul(
                                op, at[:, j, :], wo_sb[:, j, n * 512:(n + 1) * 512],
                                start=(j == 0), stop=(j == 7),
                            )
                        ob = obp.tile([128, 512], F32, tag="ob")
                        nc.vector.tensor_copy(ob, op)
                        nc.scalar.dma_start(
                            out=out_d[c * W:(c + 1) * W, n * 512:(n + 1) * 512], in_=ob)


def _make_masks(first_chunk_global: bool):
    i = np.arange(W)[None, :]      # query within chunk (free dim)
    j = np.arange(W)[:, None]      # key within block (partition dim)
    causal_t = np.where(i >= j, 0.0, MASK_NEG).astype(np.float32)       # block is CUR
    strict_t = np.where(j >= i + 1, 0.0, MASK_NEG).astype(np.float32)   # block is PREV
    v0 = np.full((W, 2 * W), MASK_NEG, np.float32)
    if not first_chunk_global:
        v0[:, W:] = strict_t
    v1 = np.concatenate([causal_t, strict_t], axis=1)
    v2 = np.full((W, 2 * W), MASK_NEG, np.float32)
    v2[:, :W] = causal_t
    return np.stack([v0, v1, v2]).astype(np.float32)


def shard_inputs(x, Wq, Wk, Wv, Wo):
    """Host-side prep: per-core input dicts."""
    wqt = np.ascontiguousarray(Wq.T.astype(np.float32))
    wkt = np.ascontiguousarray(Wk.T.astype(np.float32))
    wvt = np.ascontiguousarray(Wv.T.astype(np.float32))
    wot = np.ascontiguousarray(Wo.T.astype(ml_dtypes.bfloat16))
    in_maps = []
    for c in range(N_CORES):
        b, s0 = c // SHARDS_PER_B, (c % SHARDS_PER_B) * OWN
        xs = np.zeros((LOC, D), np.float32)
        lo = max(0, s0 - HALO)
        xs[HALO - (s0 - lo):] = x[b, lo:s0 + OWN]
        in_maps.append({
            "xt": np.ascontiguousarray(xs.T),
            "wqt": wqt, "wkt": wkt, "wvt": wvt, "wot": wot,
            "masks": _make_masks(s0 == 0),
            "ident": np.eye(128, dtype=np.float32),
        })
    return in_maps


_NC_CACHE = {}


def _get_nc(reps=1):
    if reps not in _NC_CACHE:
        _NC_CACHE[reps] = build_nc(reps)
    return _NC_CACHE[reps]


def kernel(x, Wq, Wk, Wv, Wo):
    x = np.asarray(x, dtype=np.float32)
    in_maps = shard_inputs(
        x, np.asarray(Wq, np.float32), np.asarray(Wk, np.float32),
        np.asarray(Wv, np.float32), np.asarray(Wo, np.float32))
    nc = _get_nc(1)
    try:
        res = run_bass_kernel_spmd(nc, in_maps, core_ids=list(range(N_CORES)))
    except Exception:
        # transient NRT device-state failures recover on retry
        res = run_bass_kernel_spmd(nc, in_maps, core_ids=list(range(N_CORES)))
    out = np.empty((B, S, D), np.float32)
    for c in range(N_CORES):
        b, s0 = c // SHARDS_PER_B, (c % SHARDS_PER_B) * OWN
        out[b, s0:s0 + OWN] = res.results[c]["out"]
    return out

